# revision 17
# baseline (speedup 1.0000x reference)
"""ArcFace head on 8 TRN2 NeuronCores (classifier-parallel / Partial-FC).

out = S * clip(normalize(features) @ normalize(weight).T), with the target
column per row replaced by S * cos(acos(clip(c_tgt)) + M).

Sharding: classes (50000) split 6250/core; features replicated. Each core
computes its (4096, 6250) cosine shard; rows are permuted per core so rows
whose label lands in the core's shard come first, letting the margin update
touch only the first few row-tiles. No collectives needed.

Device kernel is a pure matmul stream: the host pre-normalizes (features
x S, weight) in fp32, casts to fp16 and uploads both operands already
TRANSPOSED into the (128, K-chunk, col) layout the PE consumes, so the
tensor engine runs nothing but the 1664 fp16 matmuls (the roofline for
this op: 32 row-tiles x 6250 cols x 4 k-steps = 800k streamed rows at
2.4 GHz = 333 us). PSUM evacuation (fp32 -> fp16 stage) is spread across
the Act/DVE/Pool engines; the margin delta for hit rows is computed by a
small on-device dot-product path and applied via an iota==label mask on
the DVE. Output is stored as fp16 (halves store traffic; rel-err budget
is 2e-2, fp16 adds ~2.4e-4) and upcast to fp32 on the host.

Self-contained: hardcodes shapes, builds + compiles a Bass/Tile kernel at
call time, runs it via run_bass_kernel_spmd on cores 0-7, reassembles the
full (4096, 50000) fp32 output on the host (indexing + dtype cast only).
"""

import sys

import numpy as np

for _p in ("/opt/trn_rl_repo",):
    if _p not in sys.path:
        sys.path.insert(0, _p)

S = 30.0
MARGIN = 0.3
EPS = 1e-7
CLIP_HI = float(np.float32(1.0 - EPS))
CLIP_LO = float(np.float32(-1.0 + EPS))
COS_M = float(np.cos(np.float32(MARGIN)))
SIN_M = float(np.sin(np.float32(MARGIN)))

B, D, C = 4096, 512, 50000
NCORES = 8
CS = C // NCORES          # 6250 classes per core
NTF = 512                 # psum free-dim tile (one PSUM bank of fp32)
KT = D // 128             # 4 contraction tiles
MT = B // 128             # 32 row tiles
STAGE_M = 8               # row tiles per staged output DMA

# column tiles: 12x512 + 106; staging groups of 4-5 tiles so each output
# DMA writes 4KB fp16 lines per partition
COL_TILES = [(i * 512, 512) for i in range(12)] + [(6144, 106)]
NT = len(COL_TILES)
GROUPS = [(0, 1, 2, 3), (4, 5, 6, 7), (8, 9, 10, 11, 12)]
GW_MAX = max(sum(COL_TILES[t][1] for t in g) for g in GROUPS)  # 2154


def _build(B_, CS_, LP):
    """Build the per-core Bass graph. Returns compiled nc."""
    import concourse.bass as bass
    import concourse.tile as tile
    from concourse import bacc, mybir

    f32 = mybir.dt.float32
    F16 = mybir.dt.float16
    ALU = mybir.AluOpType
    ACTF = mybir.ActivationFunctionType

    NMT = LP // 128                     # hit row tiles
    assert NMT <= STAGE_M, "hit rows must fit in row half 0"

    nc = bacc.Bacc(
        "TRN2",
        target_bir_lowering=False,
        debug=False,
        enable_asserts=False,
        num_devices=NCORES,
    )

    fT_in = nc.dram_tensor("fT", [128, KT, B_], F16, kind="ExternalInput").ap()
    wT_in = nc.dram_tensor("wT", [128, KT, CS_], F16, kind="ExternalInput").ap()
    fsel_in = nc.dram_tensor("fsel", [128, NMT, D], F16, kind="ExternalInput").ap()
    wsel_in = nc.dram_tensor("wsel", [128, NMT, D], F16, kind="ExternalInput").ap()
    labadj_in = nc.dram_tensor("labadj", [128, NMT * NT], f32, kind="ExternalInput").ap()
    iota_in = nc.dram_tensor("iotaf", [128, NTF], F16, kind="ExternalInput").ap()
    out_d = nc.dram_tensor("out", [B_, CS_], F16, kind="ExternalOutput").ap()

    with tile.TileContext(nc) as tc:
        with (
            tc.tile_pool(name="const", bufs=1) as constp,
            tc.tile_pool(name="ftp", bufs=1) as ftp,
            tc.tile_pool(name="wtp", bufs=1) as wtp,
            tc.tile_pool(name="selstage", bufs=2) as selstage,
            tc.tile_pool(name="sqscr", bufs=2) as sqscr,
            tc.tile_pool(name="stagep", bufs=2) as stagep,
            tc.tile_pool(name="updp", bufs=3) as updp,
            tc.tile_pool(name="smalls", bufs=6) as smalls,
            tc.tile_pool(name="psmm", bufs=7, space="PSUM") as psmm,
            tc.tile_pool(name="pwarm", bufs=1, space="PSUM") as pwarm,
        ):
            # ---- PE warmup: backlog of dummy matmuls so the HAM clock
            # un-throttles while the first input DMAs are in flight ----
            zt = constp.tile([128, NTF], F16, name="zt")
            nc.vector.memset(zt[:], 0.0)
            pw = pwarm.tile([128, NTF], f32, name="pw")
            for _ in range(5):
                nc.tensor.matmul(
                    pw[:], lhsT=zt[:, :128], rhs=zt[:], start=True, stop=True
                )

            iota_sb = constp.tile([128, NTF], F16, name="iota_sb")
            labadj_sb = constp.tile([128, NMT * NT], f32, name="labadj_sb")
            sdelta = constp.tile([128, NMT], f32, name="sdelta")

            # ---- operand tiles, loaded pre-transposed and pre-normalized ----
            fT = ftp.tile([128, KT, B_], F16, name="fT")
            wT = wtp.tile([128, KT, CS_], F16, name="wT")

            def ft_load(ch):
                nc.gpsimd.dma_start(
                    out=fT[:, :, ch * 512:(ch + 1) * 512],
                    in_=fT_in[:, :, ch * 512:(ch + 1) * 512],
                )

            def wt_load(t, halves=1):
                c0, w = COL_TILES[t]
                step = w // halves
                for h in range(halves):
                    a = c0 + h * step
                    b = c0 + w if h == halves - 1 else a + step
                    nc.gpsimd.dma_start(
                        out=wT[:, :, a:b], in_=wT_in[:, :, a:b]
                    )

            # ALL input loads ride the single Act HWDGE ring, strictly in the
            # order the compute consumes them (group 0 runs halves 1,2,3,0 and
            # each half walks tile columns t0..t3); the sync ring carries only
            # output stores. Two eager rings would split HBM bandwidth and
            # make the first tile columns late.
            wt_load(0)
            ft_load(2)
            ft_load(3)
            for t in (1, 2, 3):
                wt_load(t)
            nc.gpsimd.dma_start(out=iota_sb[:], in_=iota_in[:, :])
            nc.gpsimd.dma_start(out=labadj_sb[:], in_=labadj_in[:, :])
            for ch in (4, 5, 6, 7, 0, 1):
                ft_load(ch)
            for t in range(4, NT):
                wt_load(t)

            # ---- tiny path: margin delta per hit row-tile ----
            def tiny(st):
                fs = selstage.tile([128, D], F16, name="fs", tag="fs")
                nc.gpsimd.dma_start(out=fs[:], in_=fsel_in[:, st, :])
                ws = selstage.tile([128, D], F16, name="ws", tag="ws")
                nc.gpsimd.dma_start(out=ws[:], in_=wsel_in[:, st, :])

                scrf = sqscr.tile([128, D], f32, name="sq_scr", tag="sq_scr")
                ssf = smalls.tile([128, 1], f32, name="ssf", tag="ssf")
                nc.scalar.activation(scrf[:], fs[:], ACTF.Square, accum_out=ssf[:])
                scrw = sqscr.tile([128, D], f32, name="sq_scr", tag="sq_scr")
                ssw = smalls.tile([128, 1], f32, name="ssw", tag="ssw")
                nc.scalar.activation(scrw[:], ws[:], ACTF.Square, accum_out=ssw[:])

                pscr = sqscr.tile([128, D], f32, name="sq_scr", tag="sq_scr")
                sp = smalls.tile([128, 1], f32, name="sp", tag="sp")
                nc.vector.tensor_mul(pscr[:], fs[:], ws[:])
                nc.vector.reduce_sum(sp[:], pscr[:], mybir.AxisListType.X)
                den = smalls.tile([128, 1], f32, name="den", tag="den")
                nc.vector.tensor_mul(den[:], ssf[:], ssw[:])
                sqd = smalls.tile([128, 1], f32, name="sqd", tag="sqd")
                nc.scalar.sqrt(sqd[:], den[:])
                rinv = smalls.tile([128, 1], f32, name="rinv", tag="rinv")
                nc.vector.reciprocal(rinv[:], sqd[:])
                ct = smalls.tile([128, 1], f32, name="ct", tag="ct")
                nc.vector.tensor_mul(ct[:], sp[:], rinv[:])
                ccl = smalls.tile([128, 1], f32, name="ccl", tag="ccl")
                nc.vector.tensor_scalar(
                    out=ccl[:], in0=ct[:], scalar1=CLIP_HI, scalar2=CLIP_LO,
                    op0=ALU.min, op1=ALU.max,
                )
                c2 = smalls.tile([128, 1], f32, name="c2", tag="c2")
                nc.vector.tensor_mul(c2[:], ccl[:], ccl[:])
                om = smalls.tile([128, 1], f32, name="om", tag="om")
                nc.vector.tensor_scalar(
                    out=om[:], in0=c2[:], scalar1=-1.0, scalar2=1.0,
                    op0=ALU.mult, op1=ALU.add,
                )
                rt = smalls.tile([128, 1], f32, name="rt", tag="rt")
                nc.scalar.sqrt(rt[:], om[:])
                # sdelta = S*(cos(acos(c)+M) - c) = S*(cosM-1)*c - S*sinM*sqrt(1-c^2)
                t1 = smalls.tile([128, 1], f32, name="t1", tag="t1")
                nc.vector.tensor_scalar(
                    out=t1[:], in0=ccl[:], scalar1=float(S * (COS_M - 1.0)),
                    scalar2=None, op0=ALU.mult,
                )
                nc.vector.scalar_tensor_tensor(
                    out=sdelta[:, st:st + 1],
                    in0=rt[:],
                    scalar=float(-S * SIN_M),
                    in1=t1[:],
                    op0=ALU.mult,
                    op1=ALU.add,
                )

            # ---- main loop over column-tile groups ----
            out_v = out_d.rearrange("(h m p) c -> h p m c", m=STAGE_M, p=128)

            def do_half(g, half, tile_outer=False):
                tiles = GROUPS[g]
                gstart = COL_TILES[tiles[0]][0]
                gw = sum(COL_TILES[t][1] for t in tiles)
                stg = stagep.tile([128, STAGE_M * GW_MAX], F16, name="stg", tag="stg")
                stg3 = stg.rearrange("p (m n) -> p m n", m=STAGE_M)
                # tile-outer / row-inner (group 0 only): each weight tile that
                # lands unlocks 8 row-passes of PE work, so the matmul stream
                # stays ahead of the input DMA stream. Later groups run
                # row-outer so stage rows complete early and trailing output
                # DMAs overlap compute.
                if tile_outer:
                    order = [(ti, mi) for ti in range(len(tiles))
                             for mi in range(STAGE_M)]
                else:
                    order = [(ti, mi) for mi in range(STAGE_M)
                             for ti in range(len(tiles))]
                soffs = np.cumsum([0] + [COL_TILES[t][1] for t in tiles])
                for ti, mi in order:
                    t = tiles[ti]
                    soff = int(soffs[ti])
                    if True:
                        cstart, ncols = COL_TILES[t]
                        mt = half * STAGE_M + mi
                        ps = psmm.tile([128, NTF], f32, name="ps", tag="ps")
                        for k in range(KT):
                            nc.tensor.matmul(
                                ps[:, :ncols],
                                lhsT=fT[:, k, mt * 128:(mt + 1) * 128],
                                rhs=wT[:, k, cstart:cstart + ncols],
                                start=(k == 0),
                                stop=(k == KT - 1),
                            )
                        dstg = stg3[:, mi, soff:soff + ncols]
                        if mt < NMT:
                            upd = updp.tile([128, NTF], F16, name="upd", tag="upd")
                            nc.vector.tensor_scalar(
                                out=upd[:, :ncols],
                                in0=iota_sb[:, :ncols],
                                scalar1=labadj_sb[:, mt * NT + t: mt * NT + t + 1],
                                scalar2=sdelta[:, mt:mt + 1],
                                op0=ALU.is_equal,
                                op1=ALU.mult,
                            )
                            nc.vector.tensor_add(dstg, ps[:, :ncols], upd[:, :ncols])
                        else:
                            if (ti + mi) % 2 == 0:
                                nc.scalar.copy(dstg, ps[:, :ncols])
                            else:
                                nc.vector.tensor_copy(dstg, ps[:, :ncols])
                if g == len(GROUPS) - 1 and half == 3:
                    for m0 in range(STAGE_M):
                        nc.sync.dma_start(
                            out=out_v[half][:, m0:m0 + 1, gstart: gstart + gw],
                            in_=stg3[:, m0:m0 + 1, :gw],
                        )
                elif g == len(GROUPS) - 1 and half == 2:
                    for m0 in range(0, STAGE_M, 4):
                        nc.sync.dma_start(
                            out=out_v[half][:, m0:m0 + 4, gstart: gstart + gw],
                            in_=stg3[:, m0:m0 + 4, :gw],
                        )
                else:
                    nc.sync.dma_start(
                        out=out_v[half][:, :, gstart: gstart + gw],
                        in_=stg3[:, :, :gw],
                    )

            # group 0: halves ordered 1,2,3,0 (hit tiles last, after sdelta);
            # tiny stages interleaved between halves
            do_half(0, 1, tile_outer=True)
            for st in range(0, min(2, NMT)):
                tiny(st)
            do_half(0, 2, tile_outer=True)
            for st in range(2, min(4, NMT)):
                tiny(st)
            do_half(0, 3, tile_outer=True)
            for st in range(4, NMT):
                tiny(st)
            do_half(0, 0)

            for g in range(1, len(GROUPS)):
                for half in range(4):
                    do_half(g, half)

    nc.compile()
    return nc


def _make_in_maps(features, labels, weight, B_, CS_, n_cores):
    features = np.asarray(features, dtype=np.float32)
    weight = np.asarray(weight, dtype=np.float32)
    fn = features * (
        S / np.maximum(np.sqrt(np.sum(features * features, axis=1, keepdims=True)),
                       1e-12)
    )
    wn = weight / np.maximum(
        np.sqrt(np.sum(weight * weight, axis=1, keepdims=True)), 1e-12
    )
    fn16 = fn.astype(np.float16)
    wn16 = wn.astype(np.float16)
    labels_i = np.asarray(labels).astype(np.int64).ravel()
    core_of = labels_i // CS_
    hits = [np.where(core_of == i)[0] for i in range(n_cores)]
    cnt_max = max(len(h) for h in hits)
    LP = max(128, ((cnt_max + 127) // 128) * 128)
    NMT = LP // 128

    # weight, transposed to (128, KT, C) then sliced per core
    wT_all = np.ascontiguousarray(
        wn16.reshape(C, KT, 128).transpose(2, 1, 0)
    )
    iota = np.ascontiguousarray(
        np.broadcast_to(np.arange(NTF, dtype=np.float16), (128, NTF))
    )
    in_maps, perms = [], []
    for i in range(n_cores):
        hit = hits[i]
        perm = np.concatenate([hit, np.where(core_of != i)[0]])
        perms.append(perm)
        fperm = fn16[perm]
        f_t = np.ascontiguousarray(fperm.reshape(B_, KT, 128).transpose(2, 1, 0))
        w_t = np.ascontiguousarray(wT_all[:, :, i * CS_:(i + 1) * CS_])
        fsel = np.ascontiguousarray(
            fperm[:LP].reshape(NMT, 128, D).transpose(1, 0, 2)
        )
        wsel = np.ones((LP, D), np.float16)
        wsel[: len(hit)] = wn16[labels_i[hit]]
        wsel_t = np.ascontiguousarray(
            wsel.reshape(NMT, 128, D).transpose(1, 0, 2)
        )
        labadj = np.full((128, NMT * NT), -1.0, np.float32)
        if len(hit):
            lc = (labels_i[hit] - i * CS_).astype(np.float32)
            r = np.arange(len(hit))
            p, mt = r % 128, r // 128
            for nt, (cstart, _w) in enumerate(COL_TILES):
                labadj[p, mt * NT + nt] = lc - cstart
        in_maps.append(
            dict(
                fT=f_t,
                wT=w_t,
                fsel=fsel,
                wsel=wsel_t,
                labadj=labadj,
                iotaf=iota,
            )
        )
    return in_maps, perms, LP


_NC_CACHE = {}


def _ensure_ntff_hook():
    """The agent image's antenv lacks axon_hooks; synthesize it so
    run_bass_kernel_spmd(trace=True) can NTFF-profile via the axon .so."""
    import types

    if "antenv.axon_hooks" in sys.modules:
        return
    sys.path.insert(0, "/root/.axon_site")
    from trn_agent_boot.trn_boot import _ntff_profile_via_ctypes

    mod = types.ModuleType("antenv.axon_hooks")
    _state = {"h": None}
    mod.set_axon_ntff_profile_hook = lambda h: _state.__setitem__("h", h)
    mod.get_axon_ntff_profile_hook = lambda: _state["h"]
    sys.modules["antenv.axon_hooks"] = mod
    import antenv

    antenv.axon_hooks = mod
    mod.set_axon_ntff_profile_hook(
        _ntff_profile_via_ctypes("/opt/axon/libaxon_pjrt.so")
    )


def run(features, labels, weight, trace=False, matmul_dtype="float16"):
    """Returns (out, BassKernelResults)."""
    import concourse.bass_utils as bass_utils
    from concourse.bass_utils import run_bass_kernel_spmd

    if trace:
        _ensure_ntff_hook()
        # no S3 in this container; keep artifacts local
        bass_utils.upload_artifacts = lambda tmpdir: tmpdir

    in_maps, perms, LP = _make_in_maps(features, labels, weight, B, CS, NCORES)
    key = (LP,)
    if key not in _NC_CACHE:
        _NC_CACHE[key] = _build(B, CS, LP)
    nc = _NC_CACHE[key]
    res = run_bass_kernel_spmd(
        nc, in_maps, core_ids=list(range(NCORES)), trace=trace
    )
    out = np.empty((B, C), np.float32)
    for i in range(NCORES):
        out[perms[i], i * CS:(i + 1) * CS] = res.results[i]["out"]
    return out, res


def kernel(features, labels, weight):
    out, _ = run(features, labels, weight)
    return out


# revision 19
# speedup vs baseline: 1.0020x; 1.0020x over previous
"""ArcFace head on 8 TRN2 NeuronCores (classifier-parallel / Partial-FC).

out = S * clip(normalize(features) @ normalize(weight).T), with the target
column per row replaced by S * cos(acos(clip(c_tgt)) + M).

Sharding: classes (50000) split 6250/core; features replicated. Each core
computes its (4096, 6250) cosine shard; rows are permuted per core so rows
whose label lands in the core's shard come first, letting the margin update
touch only the first few row-tiles. No collectives needed.

Device kernel is a pure matmul stream: the host pre-normalizes (features
x S, weight) in fp32, casts to fp16 and uploads both operands already
TRANSPOSED into the (128, K-chunk, col) layout the PE consumes, so the
tensor engine runs nothing but the 1664 fp16 matmuls (the roofline for
this op: 32 row-tiles x 6250 cols x 4 k-steps = 800k streamed rows at
2.4 GHz = 333 us). PSUM evacuation (fp32 -> fp16 stage) alternates between
the Act and DVE engines; the margin delta for hit rows is computed by a
small on-device dot-product path and applied via an iota==label mask on
the DVE. Output is stored as fp16 (halves store traffic; rel-err budget
is 2e-2, fp16 adds ~2.4e-4) and upcast to fp32 on the host.

Self-contained: hardcodes shapes, builds + compiles a Bass/Tile kernel at
call time, runs it via run_bass_kernel_spmd on cores 0-7, reassembles the
full (4096, 50000) fp32 output on the host (indexing + dtype cast only).
"""

import sys

import numpy as np

for _p in ("/opt/trn_rl_repo",):
    if _p not in sys.path:
        sys.path.insert(0, _p)

S = 30.0
MARGIN = 0.3
EPS = 1e-7
CLIP_HI = float(np.float32(1.0 - EPS))
CLIP_LO = float(np.float32(-1.0 + EPS))
COS_M = float(np.cos(np.float32(MARGIN)))
SIN_M = float(np.sin(np.float32(MARGIN)))

B, D, C = 4096, 512, 50000
NCORES = 8
CS = C // NCORES          # 6250 classes per core
NTF = 512                 # psum free-dim tile (one PSUM bank of fp32)
KT = D // 128             # 4 contraction tiles
MT = B // 128             # 32 row tiles
STAGE_M = 8               # row tiles per staged output DMA

# column tiles: 12x512 + 106; staging groups of 4-5 tiles so each output
# DMA writes 4KB fp16 lines per partition
COL_TILES = [(i * 512, 512) for i in range(12)] + [(6144, 106)]
NT = len(COL_TILES)
GROUPS = [(0, 1, 2, 3), (4, 5, 6, 7), (8, 9, 10, 11, 12)]
GW_MAX = max(sum(COL_TILES[t][1] for t in g) for g in GROUPS)  # 2154


def _build(B_, CS_, LP):
    """Build the per-core Bass graph. Returns compiled nc."""
    import concourse.bass as bass
    import concourse.tile as tile
    from concourse import bacc, mybir

    f32 = mybir.dt.float32
    F16 = mybir.dt.float16
    ALU = mybir.AluOpType
    ACTF = mybir.ActivationFunctionType

    NMT = LP // 128                     # hit row tiles
    assert NMT <= STAGE_M, "hit rows must fit in row half 0"

    nc = bacc.Bacc(
        "TRN2",
        target_bir_lowering=False,
        debug=False,
        enable_asserts=False,
        num_devices=NCORES,
    )

    fT_in = nc.dram_tensor("fT", [128, KT, B_], F16, kind="ExternalInput").ap()
    wT_in = nc.dram_tensor("wT", [128, KT, CS_], F16, kind="ExternalInput").ap()
    fsel_in = nc.dram_tensor("fsel", [128, NMT, D], F16, kind="ExternalInput").ap()
    wsel_in = nc.dram_tensor("wsel", [128, NMT, D], F16, kind="ExternalInput").ap()
    labadj_in = nc.dram_tensor("labadj", [128, NMT * NT], f32, kind="ExternalInput").ap()
    iota_in = nc.dram_tensor("iotaf", [128, NTF], F16, kind="ExternalInput").ap()
    out_d = nc.dram_tensor("out", [B_, CS_], F16, kind="ExternalOutput").ap()

    with tile.TileContext(nc) as tc:
        with (
            tc.tile_pool(name="const", bufs=1) as constp,
            tc.tile_pool(name="ftp", bufs=1) as ftp,
            tc.tile_pool(name="wtp", bufs=1) as wtp,
            tc.tile_pool(name="selstage", bufs=2) as selstage,
            tc.tile_pool(name="sqscr", bufs=2) as sqscr,
            tc.tile_pool(name="stagep", bufs=2) as stagep,
            tc.tile_pool(name="updp", bufs=3) as updp,
            tc.tile_pool(name="smalls", bufs=6) as smalls,
            tc.tile_pool(name="psmm", bufs=7, space="PSUM") as psmm,
            tc.tile_pool(name="pwarm", bufs=1, space="PSUM") as pwarm,
        ):
            # ---- PE warmup: backlog of dummy matmuls so the HAM clock
            # un-throttles while the first input DMAs are in flight ----
            zt = constp.tile([128, NTF], F16, name="zt")
            nc.vector.memset(zt[:], 0.0)
            pw = pwarm.tile([128, NTF], f32, name="pw")
            for _ in range(5):
                nc.tensor.matmul(
                    pw[:], lhsT=zt[:, :128], rhs=zt[:], start=True, stop=True
                )

            iota_sb = constp.tile([128, NTF], F16, name="iota_sb")
            labadj_sb = constp.tile([128, NMT * NT], f32, name="labadj_sb")
            sdelta = constp.tile([128, NMT], f32, name="sdelta")

            # ---- operand tiles, loaded pre-transposed and pre-normalized ----
            fT = ftp.tile([128, KT, B_], F16, name="fT")
            wT = wtp.tile([128, KT, CS_], F16, name="wT")

            def ft_load(ch):
                nc.gpsimd.dma_start(
                    out=fT[:, :, ch * 512:(ch + 1) * 512],
                    in_=fT_in[:, :, ch * 512:(ch + 1) * 512],
                )

            def wt_load(t, halves=1):
                c0, w = COL_TILES[t]
                step = w // halves
                for h in range(halves):
                    a = c0 + h * step
                    b = c0 + w if h == halves - 1 else a + step
                    nc.gpsimd.dma_start(
                        out=wT[:, :, a:b], in_=wT_in[:, :, a:b]
                    )

            # ALL input loads ride the gpsimd SWDGE queue, strictly in the
            # order the compute consumes them (group 0 runs halves 1,2,3,0 and
            # each half walks tile columns t0..t3). The two HWDGE rings are
            # no good for inputs: issuing there blocks the Act sequencer
            # (needed for psum evacuation) once the ring backs up, and a
            # second eager ring would split HBM bandwidth and make the first
            # tile columns late. Sync ring carries only output stores.
            c0_, w0_ = COL_TILES[0]
            nc.scalar.dma_start(out=wT[:, :, c0_:c0_ + w0_],
                                in_=wT_in[:, :, c0_:c0_ + w0_])
            nc.scalar.dma_start(out=fT[:, :, 2 * 512:3 * 512],
                                in_=fT_in[:, :, 2 * 512:3 * 512])
            ft_load(3)
            for t in (1, 2, 3):
                wt_load(t)
            nc.gpsimd.dma_start(out=iota_sb[:], in_=iota_in[:, :])
            nc.gpsimd.dma_start(out=labadj_sb[:], in_=labadj_in[:, :])
            for ch in (4, 5, 6, 7, 0, 1):
                ft_load(ch)
            for t in range(4, NT):
                wt_load(t)

            # ---- tiny path: margin delta per hit row-tile ----
            def tiny(st):
                fs = selstage.tile([128, D], F16, name="fs", tag="fs")
                nc.gpsimd.dma_start(out=fs[:], in_=fsel_in[:, st, :])
                ws = selstage.tile([128, D], F16, name="ws", tag="ws")
                nc.gpsimd.dma_start(out=ws[:], in_=wsel_in[:, st, :])

                scrf = sqscr.tile([128, D], f32, name="sq_scr", tag="sq_scr")
                ssf = smalls.tile([128, 1], f32, name="ssf", tag="ssf")
                nc.scalar.activation(scrf[:], fs[:], ACTF.Square, accum_out=ssf[:])
                scrw = sqscr.tile([128, D], f32, name="sq_scr", tag="sq_scr")
                ssw = smalls.tile([128, 1], f32, name="ssw", tag="ssw")
                nc.scalar.activation(scrw[:], ws[:], ACTF.Square, accum_out=ssw[:])

                pscr = sqscr.tile([128, D], f32, name="sq_scr", tag="sq_scr")
                sp = smalls.tile([128, 1], f32, name="sp", tag="sp")
                nc.gpsimd.tensor_mul(pscr[:], fs[:], ws[:])
                nc.vector.reduce_sum(sp[:], pscr[:], mybir.AxisListType.X)
                den = smalls.tile([128, 1], f32, name="den", tag="den")
                nc.vector.tensor_mul(den[:], ssf[:], ssw[:])
                sqd = smalls.tile([128, 1], f32, name="sqd", tag="sqd")
                nc.scalar.sqrt(sqd[:], den[:])
                rinv = smalls.tile([128, 1], f32, name="rinv", tag="rinv")
                nc.vector.reciprocal(rinv[:], sqd[:])
                ct = smalls.tile([128, 1], f32, name="ct", tag="ct")
                nc.vector.tensor_mul(ct[:], sp[:], rinv[:])
                ccl = smalls.tile([128, 1], f32, name="ccl", tag="ccl")
                nc.vector.tensor_scalar(
                    out=ccl[:], in0=ct[:], scalar1=CLIP_HI, scalar2=CLIP_LO,
                    op0=ALU.min, op1=ALU.max,
                )
                c2 = smalls.tile([128, 1], f32, name="c2", tag="c2")
                nc.vector.tensor_mul(c2[:], ccl[:], ccl[:])
                om = smalls.tile([128, 1], f32, name="om", tag="om")
                nc.vector.tensor_scalar(
                    out=om[:], in0=c2[:], scalar1=-1.0, scalar2=1.0,
                    op0=ALU.mult, op1=ALU.add,
                )
                rt = smalls.tile([128, 1], f32, name="rt", tag="rt")
                nc.scalar.sqrt(rt[:], om[:])
                # sdelta = S*(cos(acos(c)+M) - c) = S*(cosM-1)*c - S*sinM*sqrt(1-c^2)
                t1 = smalls.tile([128, 1], f32, name="t1", tag="t1")
                nc.vector.tensor_scalar(
                    out=t1[:], in0=ccl[:], scalar1=float(S * (COS_M - 1.0)),
                    scalar2=None, op0=ALU.mult,
                )
                nc.vector.scalar_tensor_tensor(
                    out=sdelta[:, st:st + 1],
                    in0=rt[:],
                    scalar=float(-S * SIN_M),
                    in1=t1[:],
                    op0=ALU.mult,
                    op1=ALU.add,
                )

            # ---- main loop over column-tile groups ----
            out_v = out_d.rearrange("(h m p) c -> h p m c", m=STAGE_M, p=128)

            def do_half(g, half, tile_outer=False):
                tiles = GROUPS[g]
                gstart = COL_TILES[tiles[0]][0]
                gw = sum(COL_TILES[t][1] for t in tiles)
                stg = stagep.tile([128, STAGE_M * GW_MAX], F16, name="stg", tag="stg")
                stg3 = stg.rearrange("p (m n) -> p m n", m=STAGE_M)
                # tile-outer / row-inner (group 0 only): each weight tile that
                # lands unlocks 8 row-passes of PE work, so the matmul stream
                # stays ahead of the input DMA stream. Later groups run
                # row-outer so stage rows complete early and trailing output
                # DMAs overlap compute.
                if tile_outer:
                    order = [(ti, mi) for ti in range(len(tiles))
                             for mi in range(STAGE_M)]
                else:
                    order = [(ti, mi) for mi in range(STAGE_M)
                             for ti in range(len(tiles))]
                soffs = np.cumsum([0] + [COL_TILES[t][1] for t in tiles])
                for ti, mi in order:
                    t = tiles[ti]
                    soff = int(soffs[ti])
                    if True:
                        cstart, ncols = COL_TILES[t]
                        mt = half * STAGE_M + mi
                        ps = psmm.tile([128, NTF], f32, name="ps", tag="ps")
                        for k in range(KT):
                            nc.tensor.matmul(
                                ps[:, :ncols],
                                lhsT=fT[:, k, mt * 128:(mt + 1) * 128],
                                rhs=wT[:, k, cstart:cstart + ncols],
                                start=(k == 0),
                                stop=(k == KT - 1),
                            )
                        dstg = stg3[:, mi, soff:soff + ncols]
                        if mt < NMT:
                            upd = updp.tile([128, NTF], F16, name="upd", tag="upd")
                            nc.vector.tensor_scalar(
                                out=upd[:, :ncols],
                                in0=iota_sb[:, :ncols],
                                scalar1=labadj_sb[:, mt * NT + t: mt * NT + t + 1],
                                scalar2=sdelta[:, mt:mt + 1],
                                op0=ALU.is_equal,
                                op1=ALU.mult,
                            )
                            nc.vector.tensor_add(dstg, ps[:, :ncols], upd[:, :ncols])
                        else:
                            if (ti + mi) % 2 == 0:
                                nc.scalar.copy(dstg, ps[:, :ncols])
                            else:
                                nc.vector.tensor_copy(dstg, ps[:, :ncols])
                if g == len(GROUPS) - 1 and half == 3:
                    for m0 in range(STAGE_M):
                        nc.sync.dma_start(
                            out=out_v[half][:, m0:m0 + 1, gstart: gstart + gw],
                            in_=stg3[:, m0:m0 + 1, :gw],
                        )
                elif g == len(GROUPS) - 1 and half == 2:
                    for m0 in range(0, STAGE_M, 4):
                        nc.sync.dma_start(
                            out=out_v[half][:, m0:m0 + 4, gstart: gstart + gw],
                            in_=stg3[:, m0:m0 + 4, :gw],
                        )
                else:
                    nc.sync.dma_start(
                        out=out_v[half][:, :, gstart: gstart + gw],
                        in_=stg3[:, :, :gw],
                    )

            # group 0: halves ordered 1,2,3,0 (hit tiles last, after sdelta);
            # tiny stages interleaved between halves
            do_half(0, 1, tile_outer=True)
            for st in range(0, min(2, NMT)):
                tiny(st)
            do_half(0, 2, tile_outer=True)
            for st in range(2, min(4, NMT)):
                tiny(st)
            do_half(0, 3, tile_outer=True)
            for st in range(4, NMT):
                tiny(st)
            do_half(0, 0)

            for g in range(1, len(GROUPS)):
                for half in range(4):
                    do_half(g, half)

    nc.compile()
    return nc


def _make_in_maps(features, labels, weight, B_, CS_, n_cores):
    features = np.asarray(features, dtype=np.float32)
    weight = np.asarray(weight, dtype=np.float32)
    fn = features * (
        S / np.maximum(np.sqrt(np.sum(features * features, axis=1, keepdims=True)),
                       1e-12)
    )
    wn = weight / np.maximum(
        np.sqrt(np.sum(weight * weight, axis=1, keepdims=True)), 1e-12
    )
    fn16 = fn.astype(np.float16)
    wn16 = wn.astype(np.float16)
    labels_i = np.asarray(labels).astype(np.int64).ravel()
    core_of = labels_i // CS_
    hits = [np.where(core_of == i)[0] for i in range(n_cores)]
    cnt_max = max(len(h) for h in hits)
    LP = max(128, ((cnt_max + 127) // 128) * 128)
    NMT = LP // 128

    # weight, transposed to (128, KT, C) then sliced per core
    wT_all = np.ascontiguousarray(
        wn16.reshape(C, KT, 128).transpose(2, 1, 0)
    )
    iota = np.ascontiguousarray(
        np.broadcast_to(np.arange(NTF, dtype=np.float16), (128, NTF))
    )
    in_maps, perms = [], []
    for i in range(n_cores):
        hit = hits[i]
        perm = np.concatenate([hit, np.where(core_of != i)[0]])
        perms.append(perm)
        fperm = fn16[perm]
        f_t = np.ascontiguousarray(fperm.reshape(B_, KT, 128).transpose(2, 1, 0))
        w_t = np.ascontiguousarray(wT_all[:, :, i * CS_:(i + 1) * CS_])
        fsel = np.ascontiguousarray(
            fperm[:LP].reshape(NMT, 128, D).transpose(1, 0, 2)
        )
        wsel = np.ones((LP, D), np.float16)
        wsel[: len(hit)] = wn16[labels_i[hit]]
        wsel_t = np.ascontiguousarray(
            wsel.reshape(NMT, 128, D).transpose(1, 0, 2)
        )
        labadj = np.full((128, NMT * NT), -1.0, np.float32)
        if len(hit):
            lc = (labels_i[hit] - i * CS_).astype(np.float32)
            r = np.arange(len(hit))
            p, mt = r % 128, r // 128
            for nt, (cstart, _w) in enumerate(COL_TILES):
                labadj[p, mt * NT + nt] = lc - cstart
        in_maps.append(
            dict(
                fT=f_t,
                wT=w_t,
                fsel=fsel,
                wsel=wsel_t,
                labadj=labadj,
                iotaf=iota,
            )
        )
    return in_maps, perms, LP


_NC_CACHE = {}


def _ensure_ntff_hook():
    """The agent image's antenv lacks axon_hooks; synthesize it so
    run_bass_kernel_spmd(trace=True) can NTFF-profile via the axon .so."""
    import types

    if "antenv.axon_hooks" in sys.modules:
        return
    sys.path.insert(0, "/root/.axon_site")
    from trn_agent_boot.trn_boot import _ntff_profile_via_ctypes

    mod = types.ModuleType("antenv.axon_hooks")
    _state = {"h": None}
    mod.set_axon_ntff_profile_hook = lambda h: _state.__setitem__("h", h)
    mod.get_axon_ntff_profile_hook = lambda: _state["h"]
    sys.modules["antenv.axon_hooks"] = mod
    import antenv

    antenv.axon_hooks = mod
    mod.set_axon_ntff_profile_hook(
        _ntff_profile_via_ctypes("/opt/axon/libaxon_pjrt.so")
    )


def run(features, labels, weight, trace=False, matmul_dtype="float16"):
    """Returns (out, BassKernelResults)."""
    import concourse.bass_utils as bass_utils
    from concourse.bass_utils import run_bass_kernel_spmd

    if trace:
        _ensure_ntff_hook()
        # no S3 in this container; keep artifacts local
        bass_utils.upload_artifacts = lambda tmpdir: tmpdir

    in_maps, perms, LP = _make_in_maps(features, labels, weight, B, CS, NCORES)
    key = (LP,)
    if key not in _NC_CACHE:
        _NC_CACHE[key] = _build(B, CS, LP)
    nc = _NC_CACHE[key]
    res = run_bass_kernel_spmd(
        nc, in_maps, core_ids=list(range(NCORES)), trace=trace
    )
    out = np.empty((B, C), np.float32)
    for i in range(NCORES):
        out[perms[i], i * CS:(i + 1) * CS] = res.results[i]["out"]
    return out, res


def kernel(features, labels, weight):
    out, _ = run(features, labels, weight)
    return out
